# revision 33
# baseline (speedup 1.0000x reference)
"""ChamferLoss Trainium2 kernel (one point cloud per NeuronCore), v2.

Per core, for its 2048-point cloud (P=2048, 16 pred chunks of 128):
- PE computes neg_d2 = 2xy-|y|^2 into ONE [128,2048] PSUM tile (4 matmuls,
  augmented K=16 fp16 hi/lo split); ACT fuses the -|x|^2 bias in a single
  2048-wide PSUM->SBUF fp16 cast (one 185ns init instead of four).
- Row direction (pred->target): fp16 halves pre-max (t1), then the custom
  DVE op ARGMAX_PACK_ANT ORs the 11-bit column index into the mantissa
  bits fp16->fp32 conversion leaves zero and max-accumulates: value+argmax
  in one pass. ttab2 interleaves rows j and j+1024 (plus the A/B columns
  inside each row) so ONE per-chunk indirect DMA returns both argmax
  candidates (HW SWDGE honors only one index per partition per gather);
  the exact fp32 recompute picks, with the A+B stages fused into single
  wide DVE ops over the interleaved layout.
- Column direction (target->pred): running DVE tensor_tensor max over
  chunks 0..13 into cm; a partial GPSIMD partition_all_reduce on cm starts
  while the loop finishes; chunks 14/15 are pair-maxed in halves right
  after cast15 (ACT runs far ahead of DVE) and partition-reduced in two
  pipelined half-width all_reduces; the rowC=max(rowA,rowB) halves run at
  2x on DVE with the free-axis sums on the otherwise-idle ACT accumulator,
  landing in separate fin columns.
- Per-partition partials [dist_x, ysum halves, feat_sq] are DMA'd out as
  a raw [128,4] fin tile; the host does the final partition sums and
  combines the 8 cores into (loss, coord_loss, feat_loss).

Structural notes: TRN2 instructions carry at most ONE semaphore wait
(bacc splits extras into EVSEM chains); inputs are consolidated into
single DMAs and cheap per-engine "observer" ops absorb cross-engine
deps to keep waits at one. The first SWDGE indirect descriptor reads a
stale offset on HW, so a sacrificial dummy gather runs first.
"""

import numpy as np

import concourse.bass as bass
import concourse.bacc as bacc
import concourse.mybir as mybir
import concourse.tile as tile
from concourse.bass_utils import run_bass_kernel_spmd
from concourse.bass_isa import ReduceOp as _ReduceOp
from concourse import dve_ops as _dve_ops
from concourse.dve_spec import (
    AluOp as _AluOp,
    Bin as _Bin,
    C0 as _C0,
    C1 as _C1,
    Spec as _Spec,
    Src0 as _Src0,
    Src1 as _Src1,
    maxx as _maxx,
)

IDX_MASK_BITS = 0x7FF
IDX_MASK_F = float(np.uint32(IDX_MASK_BITS).view(np.float32))
NEG_HUGE = -3.0e38


def _ref_argmax_pack(in0, in1, c0, c1, c2):
    # packed = bits(fp32(in0)) | (bits(in1) & bits(c0)); accum = row max
    v = np.asarray(in0, np.float32)
    np_ = v.shape[0]
    vb = v.view(np.uint32).reshape(np_, -1)
    ib = np.asarray(in1, np.float32).view(np.uint32).reshape(np_, -1)
    c0f = np.float32(c0.flat[0] if isinstance(c0, np.ndarray) else c0)
    c1f = np.float32(c1.flat[0] if isinstance(c1, np.ndarray) else c1)
    mask = c0f.view(np.uint32)
    packed = (vb | (ib & mask)).view(np.float32)
    acc = np.maximum(packed.max(axis=-1, keepdims=True), c1f)
    return packed, acc


ARGMAX_PACK_ANT = _dve_ops.DveOp(
    "ARGMAX_PACK_ANT",
    _Spec(
        body=_Bin(_AluOp.BITWISE_OR, _Src0, _Bin(_AluOp.BITWISE_AND, _Src1, _C0)),
        accum=_maxx,
        accum_init=_C1,
        reference=_ref_argmax_pack,
    ),
    subdim=False,
    uops_sha={"v3": "1ec944e8e2fafb91", "v4": "a87bc82f01e7f970"},
)
if ARGMAX_PACK_ANT.name not in _dve_ops._SUB_OPCODE_FOR_NAME:
    _dve_ops.OPS.append(ARGMAX_PACK_ANT)
    _dve_ops.CUSTOM_DVE_SPECS[ARGMAX_PACK_ANT.name] = ARGMAX_PACK_ANT.spec
    _dve_ops._SUB_OPCODE_FOR_NAME[ARGMAX_PACK_ANT.name] = (
        max(_dve_ops._SUB_OPCODE_FOR_NAME.values()) + 1
    )

B = 8          # clouds
P = 2048       # points per cloud
DF = 16        # feature dim
NCH = P // 128   # 16 pred chunks of 128

f16 = mybir.dt.float16
f32 = mybir.dt.float32
u32 = mybir.dt.uint32

# ptabs column layout (x and pf duplicated so the A+B candidate
# recompute stages are single wide DVE ops against interleaved gall)
XC0 = 0                    # [128, 6*NCH] pred coords, duplicated [x|x]
PF0 = 6 * NCH              # [128, 2*DF*NCH] pred feats, duplicated [pf|pf]
NX0 = PF0 + 2 * DF * NCH   # [128, NCH] -|x|^2
PTW = NX0 + NCH            # total width (624)

_CACHED = {}


def _build_nc():
    nc = bacc.Bacc("TRN2", target_bir_lowering=False, debug=False, num_devices=B)

    xyaug = nc.dram_tensor("xyaug", [16, 2 * P], f16, kind="ExternalInput").ap()
    ptabs = nc.dram_tensor("ptabs", [128, PTW], f32, kind="ExternalInput").ap()
    ttab2 = nc.dram_tensor("ttab2", [P // 2, 40], f32, kind="ExternalInput").ap()
    iob = nc.dram_tensor("iob", [128, P // 2], f32, kind="ExternalInput").ap()
    res = nc.dram_tensor("res", [128, 4], f32, kind="ExternalOutput").ap()

    AL = mybir.AluOpType
    ACTF = mybir.ActivationFunctionType

    with tile.TileContext(nc) as tc:
        with (
            tc.tile_pool(name="const", bufs=1) as cpool,
            tc.tile_pool(name="d2", bufs=NCH) as d2pool,
            tc.tile_pool(name="tree", bufs=2) as tpool,
            tc.tile_pool(name="psmm", bufs=2, space="PSUM") as psmm,
        ):
            xyaug_s = cpool.tile([16, 2 * P], f16, tag="xyaug")
            ptabs_s = cpool.tile([128, PTW], f32, tag="ptabs")
            cm = cpool.tile([128, P], f16, tag="cm")
            m2 = cpool.tile([128, P], f16, tag="m2")
            rowA = cpool.tile([128, P], f16, tag="rowA")
            rowB = cpool.tile([128, P], f16, tag="rowB")
            rowC = cpool.tile([1, P], f16, tag="rowC")
            rjunk = cpool.tile([1, P], f16, tag="rjunk")
            iob_s = cpool.tile([128, P // 2], f32, tag="iob")
            packed_all = cpool.tile([128, NCH], f32, tag="packed")
            gall = cpool.tile([128, 40 * NCH], f32, tag="gall")
            cd = cpool.tile([128, 6 * NCH], f32, tag="cd")
            fd = cpool.tile([128, 2 * DF * NCH], f32, tag="fd")
            dmin2 = cpool.tile([128, 2 * NCH], f32, tag="dmin2")
            fsq2 = cpool.tile([128, 2 * NCH], f32, tag="fsq2")
            dmin = cpool.tile([128, NCH], f32, tag="dmin")
            fselA = cpool.tile([128, NCH], f32, tag="fselA")
            fmask = cpool.tile([128, NCH], f32, tag="fmask")
            idx2 = cpool.tile([128, NCH], u32, tag="idx2")
            dx = cpool.tile([128, 1], f32, tag="dx")
            df_ = cpool.tile([128, 1], f32, tag="df")
            fin = cpool.tile([128, 4], f32, tag="fin")
            junk_d = cpool.tile([128, 1], f32, tag="junk_d")
            junk_a = cpool.tile([128, 1], f32, tag="junk_a")
            idx0 = cpool.tile([128, 1], u32, tag="idx0")
            gjunk = cpool.tile([128, 40], f32, tag="gjunk")

            # --- input loads (single DMA each => single HW queue each) ---
            nc.sync.dma_start(xyaug_s[:, :], xyaug[:, :])
            nc.sync.dma_start(ptabs_s[:, :], ptabs[:, :])
            nc.sync.dma_start(iob_s[:, :], iob[:, :])
            nc.vector.memset(fin[:, :], 0.0)

            # sacrificial first indirect DMA: the first SWDGE descriptor
            # reads a stale offset on HW, so burn it on a dummy gather.
            nc.vector.memset(idx0[:, :], 0)
            nc.gpsimd.indirect_dma_start(
                out=gjunk[:, :],
                out_offset=None,
                in_=ttab2[:, :],
                in_offset=bass.IndirectOffsetOnAxis(ap=idx0[:, :], axis=0),
            )

            # observers: absorb input-DMA deps one engine at a time
            nc.vector.tensor_copy(out=junk_d[:, :], in_=ptabs_s[:, 0:1])
            nc.vector.tensor_copy(out=junk_d[:, :], in_=iob_s[:, 0:1])
            nc.scalar.activation(
                junk_a[:, :], ptabs_s[:, NX0 : NX0 + 1], ACTF.Copy, bias=0.0, scale=1.0
            )

            xc6 = ptabs_s[:, XC0 : XC0 + 6 * NCH].rearrange("p (c k) -> p c k", k=6)
            pf6 = ptabs_s[:, PF0 : PF0 + 2 * DF * NCH].rearrange(
                "p (c k) -> p c k", k=2 * DF
            )
            g3 = gall[:, :].rearrange("p (c k) -> p c k", k=40)
            cd6 = cd[:, :].rearrange("p (c k) -> p c k", k=6)
            cd4 = cd[:, :].rearrange("p (c n k) -> p c n k", n=2, k=3)
            dm3 = dmin2[:, :].rearrange("p (c n) -> p c n", n=2)
            fd6 = fd[:, :].rearrange("p (c k) -> p c k", k=2 * DF)
            fd4 = fd[:, :].rearrange("p (c n k) -> p c n k", n=2, k=DF)
            fq3 = fsq2[:, :].rearrange("p (c n) -> p c n", n=2)

            def recompute(lo, hi):
                sl = slice(lo, hi)
                nc.vector.tensor_tensor(
                    out=cd6[:, sl], in0=xc6[:, sl], in1=g3[:, sl, 0:6],
                    op=AL.subtract,
                )
                nc.vector.tensor_tensor(
                    out=cd[:, 6 * lo : 6 * hi], in0=cd[:, 6 * lo : 6 * hi],
                    in1=cd[:, 6 * lo : 6 * hi], op=AL.mult,
                )
                nc.vector.tensor_reduce(
                    out=dm3[:, sl], in_=cd4[:, sl],
                    axis=mybir.AxisListType.X, op=AL.add,
                )
                nc.vector.tensor_tensor(
                    out=fd6[:, sl], in0=pf6[:, sl], in1=g3[:, sl, 6 : 6 + 2 * DF],
                    op=AL.subtract,
                )
                nc.vector.tensor_tensor(
                    out=fd[:, 2 * DF * lo : 2 * DF * hi],
                    in0=fd[:, 2 * DF * lo : 2 * DF * hi],
                    in1=fd[:, 2 * DF * lo : 2 * DF * hi], op=AL.mult,
                )
                nc.vector.tensor_reduce(
                    out=fq3[:, sl], in_=fd4[:, sl],
                    axis=mybir.AxisListType.X, op=AL.add,
                )


            # --- main loop over pred chunks ---
            for c in range(NCH):
                ps = psmm.tile([128, P], f32, tag="mm")
                for t in range(4):
                    nc.tensor.matmul(
                        ps[:, 512 * t : 512 * (t + 1)],
                        lhsT=xyaug_s[:, bass.ts(c, 128)],
                        rhs=xyaug_s[:, P + 512 * t : P + 512 * (t + 1)],
                        start=True,
                        stop=True,
                    )
                d2c = d2pool.tile([128, P], f16, tag="d2")
                # neg_d2 = (2xy - |y|^2) - |x|^2, one wide cast to fp16
                nc.scalar.activation(
                    d2c[:, :],
                    ps[:, :],
                    ACTF.Identity,
                    bias=ptabs_s[:, NX0 + c : NX0 + c + 1],
                    scale=1.0,
                )
                if c == 15:
                    # m2 halves RIGHT AFTER cast15 (ACT runs far ahead of
                    # DVE) so the AR#2 halves clear Pool before the last
                    # gather needs it
                    d2c14 = d2c14_ref[0]
                    nc.vector.tensor_tensor(
                        out=m2[:, 0:1024], in0=d2c14[:, 0:1024],
                        in1=d2c[:, 0:1024], op=AL.max,
                    )
                    nc.gpsimd.partition_all_reduce(
                        rowB[:, 0:1024], m2[:, 0:1024], 128, _ReduceOp.max
                    )
                    nc.vector.tensor_tensor(
                        out=m2[:, 1024:2048], in0=d2c14[:, 1024:2048],
                        in1=d2c[:, 1024:2048], op=AL.max,
                    )
                    nc.gpsimd.partition_all_reduce(
                        rowB[:, 1024:2048], m2[:, 1024:2048], 128, _ReduceOp.max
                    )
                # pre-reduce halves at 2x, then packed argmax over 1024
                # cols; the j / j+1024 ambiguity is resolved by gathering
                # BOTH candidates (interleaved in one ttab2 row) and
                # letting the exact fp32 recompute pick.
                t1 = tpool.tile([128, 1024], f16, tag="t1")
                nc.vector.tensor_tensor(
                    out=t1[:, :], in0=d2c[:, 0:1024], in1=d2c[:, 1024:2048], op=AL.max
                )
                pk = tpool.tile([128, 1024], f32, tag="pk")
                nc.vector._custom_dve(
                    ARGMAX_PACK_ANT,
                    out=pk[:, :],
                    in0=t1[:, :],
                    in1=iob_s[:, :],
                    s0=IDX_MASK_F,
                    s1=NEG_HUGE,
                    accum_out=packed_all[:, c : c + 1],
                )
                # column-direction running elementwise max: chunks 0..13
                # accumulate into cm (partition-reduced early, while the
                # loop tail runs); 14/15 into m2 (reduced separately).
                if c == 0:
                    nc.vector.tensor_copy(out=cm[:, :], in_=d2c[:, :])
                elif c <= 13:
                    nc.vector.tensor_tensor(
                        out=cm[:, :], in0=cm[:, :], in1=d2c[:, :], op=AL.max
                    )
                elif c == 14:
                    d2c14_ref = [d2c]

                if c == 13:
                    nc.gpsimd.partition_all_reduce(
                        rowA[:, :], cm[:, :], 128, _ReduceOp.max
                    )
                # per-chunk index extraction + single gather (ttab2 rows
                # carry BOTH candidates; HW SWDGE honors one index per
                # partition per gather, so batching across chunks is out)
                nc.vector.tensor_scalar(
                    out=idx2[:, c : c + 1],
                    in0=packed_all[:, c : c + 1].bitcast(u32),
                    scalar1=IDX_MASK_BITS,
                    scalar2=None,
                    op0=AL.bitwise_and,
                )
                nc.gpsimd.indirect_dma_start(
                    out=gall[:, 40 * c : 40 * (c + 1)],
                    out_offset=None,
                    in_=ttab2[:, :],
                    in_offset=bass.IndirectOffsetOnAxis(ap=idx2[:, c : c + 1], axis=0),
                )

            # --- x-direction: recompute BOTH candidates in fp32, pick min.
            # ttab2 interleaves candidate columns (yA|yB|tfA|tfB) and ptabs
            # duplicates x/pf, so each A+B stage is ONE wide DVE op. Split
            # into chunks 0..13 / 14..15 so the big half doesn't wait for
            # the last gather, and interleave the rowC/ysum halves (each
            # waits on its AR#2 half) between recompute stages.
            recompute(0, 12)
            # rowC halves at 2x; free-axis sums on the idle ACT engine, the
            # two half-sums land on different fin partitions so the final
            # partition-reducing matmul adds them for free.
            nc.vector.tensor_tensor(
                out=rowC[:, 0:1024], in0=rowA[0:1, 0:1024],
                in1=rowB[0:1, 0:1024], op=AL.max,
            )
            nc.scalar.activation(
                rjunk[:, 0:1024], rowC[:, 0:1024], ACTF.Identity,
                bias=0.0, scale=1.0, accum_out=fin[0:1, 1:2],
            )
            nc.vector.tensor_tensor(
                out=rowC[:, 1024:2048], in0=rowA[0:1, 1024:2048],
                in1=rowB[0:1, 1024:2048], op=AL.max,
            )
            nc.scalar.activation(
                rjunk[:, 1024:2048], rowC[:, 1024:2048], ACTF.Identity,
                bias=0.0, scale=1.0, accum_out=fin[0:1, 3:4],
            )
            recompute(12, 16)
            distA, distB = dm3[:, :, 0:1], dm3[:, :, 1:2]
            fqA, fqB = fq3[:, :, 0:1], fq3[:, :, 1:2]
            nc.vector.tensor_reduce(
                out=dmin[:, :], in_=dm3, axis=mybir.AxisListType.X, op=AL.min
            )
            nc.vector.tensor_reduce(
                out=dx[:, :], in_=dmin[:, :], axis=mybir.AxisListType.X, op=AL.add
            )
            # fsel = fsqB + (fsqA - fsqB) * (distA <= distB)
            fmask3 = fmask[:, :].rearrange("p (c n) -> p c n", n=1)
            fselA3 = fselA[:, :].rearrange("p (c n) -> p c n", n=1)
            nc.vector.tensor_tensor(out=fmask3, in0=distA, in1=distB, op=AL.is_le)
            nc.vector.tensor_tensor(out=fselA3, in0=fqA, in1=fqB, op=AL.subtract)
            nc.vector.tensor_tensor(
                out=fselA[:, :], in0=fselA[:, :], in1=fmask[:, :], op=AL.mult
            )
            nc.vector.tensor_tensor(out=fselA3, in0=fselA3, in1=fqB, op=AL.add)
            nc.vector.tensor_reduce(
                out=df_[:, :], in_=fselA[:, :], axis=mybir.AxisListType.X, op=AL.add
            )

            # --- stack per-partition partials; host does the final sums ---
            nc.vector.tensor_copy(out=fin[:, 0:1], in_=dx[:, :])
            nc.vector.tensor_copy(out=fin[:, 2:3], in_=df_[:, :])
            nc.sync.dma_start(res[:, :], fin[:, :])

    nc.compile()
    return nc


def _prep_core(x, y, pf, tf):
    """Host-side layout prep for one cloud (dtype splits / transposes)."""
    x = np.ascontiguousarray(x, np.float32)
    y = np.ascontiguousarray(y, np.float32)
    xh = x.astype(np.float16)
    xl = (x - xh.astype(np.float32)).astype(np.float16)
    yh = y.astype(np.float16)
    yl = (y - yh.astype(np.float32)).astype(np.float16)

    ny2 = (y.astype(np.float64) ** 2).sum(1)
    a0 = (-ny2).astype(np.float16)
    r = -ny2 - a0.astype(np.float64)
    a1 = r.astype(np.float16)
    a2 = (r - a1.astype(np.float64)).astype(np.float16)

    xyaug = np.zeros((16, 2 * P), np.float16)
    for k in range(3):
        txh = (xh[:, k].astype(np.float32) * 2).astype(np.float16)
        txl = (xl[:, k].astype(np.float32) * 2).astype(np.float16)
        xyaug[4 * k + 0, :P] = txh
        xyaug[4 * k + 1, :P] = txh
        xyaug[4 * k + 2, :P] = txl
        xyaug[4 * k + 3, :P] = txl
        xyaug[4 * k + 0, P:] = yh[:, k]
        xyaug[4 * k + 1, P:] = yl[:, k]
        xyaug[4 * k + 2, P:] = yh[:, k]
        xyaug[4 * k + 3, P:] = yl[:, k]
    xyaug[12:15, :P] = np.float16(1.0)
    xyaug[12, P:] = a0
    xyaug[13, P:] = a1
    xyaug[14, P:] = a2

    nx2 = (x.astype(np.float64) ** 2).sum(1).astype(np.float32)

    ptabs = np.zeros((128, PTW), np.float32)
    xcd = x.reshape(NCH, 128, 3)
    ptabs[:, XC0 : XC0 + 6 * NCH] = (
        np.concatenate([xcd, xcd], axis=2).transpose(1, 0, 2).reshape(128, 6 * NCH)
    )
    pfc = np.asarray(pf, np.float32).reshape(NCH, 128, DF)
    ptabs[:, PF0 : PF0 + 2 * DF * NCH] = (
        np.concatenate([pfc, pfc], axis=2)
        .transpose(1, 0, 2).reshape(128, 2 * DF * NCH)
    )
    ptabs[:, NX0 : NX0 + NCH] = (-nx2).reshape(NCH, 128).T

    # interleaved candidate table: row j = [y_j | y_{j+1024} | tf_j |
    # tf_{j+1024} | pad] so one gather returns both argmax candidates and
    # the A+B recompute stages are single wide ops
    ttab2 = np.zeros((P // 2, 40), np.float32)
    ttab2[:, 0:3] = y[: P // 2]
    ttab2[:, 3:6] = y[P // 2 :]
    ttab2[:, 6 : 6 + DF] = tf[: P // 2]
    ttab2[:, 6 + DF : 6 + 2 * DF] = tf[P // 2 :]

    iob = (np.uint32(0x3F800000) | np.arange(P // 2, dtype=np.uint32)).view(
        np.float32
    )
    iob = np.broadcast_to(iob, (128, P // 2)).copy()
    return {"xyaug": xyaug, "ptabs": ptabs, "ttab2": ttab2, "iob": iob}


def kernel(pred_coord, target_coord, pred_feat, target_feat,
           pred_offset, target_offset):
    pred_offset = np.asarray(pred_offset)
    target_offset = np.asarray(target_offset)
    starts_p = np.concatenate([[0], pred_offset[:-1]])
    starts_t = np.concatenate([[0], target_offset[:-1]])
    assert np.all(pred_offset - starts_p == P), "kernel hardcodes equal segments"
    assert np.all(target_offset - starts_t == P), "kernel hardcodes equal segments"

    if "nc" not in _CACHED:
        _CACHED["nc"] = _build_nc()
    nc = _CACHED["nc"]

    in_maps = []
    for b in range(B):
        sp, st = int(starts_p[b]), int(starts_t[b])
        in_maps.append(
            _prep_core(
                np.asarray(pred_coord)[sp : sp + P],
                np.asarray(target_coord)[st : st + P],
                np.asarray(pred_feat)[sp : sp + P],
                np.asarray(target_feat)[st : st + P],
            )
        )

    out = run_bass_kernel_spmd(nc, in_maps, core_ids=list(range(B)))
    rs = np.stack([out.results[b]["res"] for b in range(B)])  # [B, 128, 4]

    sum_x = rs[:, :, 0].sum(1)  # per-cloud sum of recomputed nearest dists
    sum_y = -(rs[:, 0, 1] + rs[:, 0, 3])  # min-dist sum (tgt->pred), negated halves
    sum_f = rs[:, :, 2].sum(1)  # per-cloud sum of squared feature diffs

    cham_x = sum_x / np.float32(P)
    cham_y = sum_y / np.float32(P)
    coord_loss = np.float32((cham_x + cham_y).sum() / B)
    feat_loss = np.float32(sum_f.sum() / (B * P * DF))
    loss = np.float32(1.0) * (np.float32(1.0) * coord_loss + np.float32(0.1) * feat_loss)
    return (np.float32(loss), np.float32(coord_loss), np.float32(feat_loss))


# revision 34
# speedup vs baseline: 1.0035x; 1.0035x over previous
"""ChamferLoss Trainium2 kernel (one point cloud per NeuronCore), v2.

Per core, for its 2048-point cloud (P=2048, 16 pred chunks of 128):
- PE computes neg_d2 = 2xy-|y|^2 into ONE [128,2048] PSUM tile (4 matmuls,
  augmented K=16 fp16 hi/lo split); ACT fuses the -|x|^2 bias in a single
  2048-wide PSUM->SBUF fp16 cast (one 185ns init instead of four).
- Row direction (pred->target): fp16 halves pre-max (t1), then the custom
  DVE op ARGMAX_PACK_ANT ORs the 11-bit column index into the mantissa
  bits fp16->fp32 conversion leaves zero and max-accumulates: value+argmax
  in one pass. ttab2 interleaves rows j and j+1024 (plus the A/B columns
  inside each row) so ONE per-chunk indirect DMA returns both argmax
  candidates (HW SWDGE honors only one index per partition per gather);
  the exact fp32 recompute picks, with the A+B stages fused into single
  wide DVE ops over the interleaved layout.
- Column direction (target->pred): running DVE tensor_tensor max over
  chunks 0..13 into cm; a partial GPSIMD partition_all_reduce on cm starts
  while the loop finishes; chunks 14/15 are pair-maxed in halves right
  after cast15 (ACT runs far ahead of DVE) and partition-reduced in two
  pipelined half-width all_reduces; the rowC=max(rowA,rowB) halves run at
  2x on DVE with the free-axis sums on the otherwise-idle ACT accumulator,
  landing in separate fin columns.
- Per-partition partials [dist_x, ysum halves, feat_sq] are DMA'd out as
  a raw [128,4] fin tile; the host does the final partition sums and
  combines the 8 cores into (loss, coord_loss, feat_loss).

Structural notes: TRN2 instructions carry at most ONE semaphore wait
(bacc splits extras into EVSEM chains); inputs are consolidated into
single DMAs and cheap per-engine "observer" ops absorb cross-engine
deps to keep waits at one. The first SWDGE indirect descriptor reads a
stale offset on HW, so a sacrificial dummy gather runs first.
"""

import numpy as np

import concourse.bass as bass
import concourse.bacc as bacc
import concourse.mybir as mybir
import concourse.tile as tile
from concourse.bass_utils import run_bass_kernel_spmd
from concourse.bass_isa import ReduceOp as _ReduceOp
from concourse import dve_ops as _dve_ops
from concourse.dve_spec import (
    AluOp as _AluOp,
    Bin as _Bin,
    C0 as _C0,
    C1 as _C1,
    Spec as _Spec,
    Src0 as _Src0,
    Src1 as _Src1,
    maxx as _maxx,
)

IDX_MASK_BITS = 0x7FF
IDX_MASK_F = float(np.uint32(IDX_MASK_BITS).view(np.float32))
NEG_HUGE = -3.0e38


def _ref_argmax_pack(in0, in1, c0, c1, c2):
    # packed = bits(fp32(in0)) | (bits(in1) & bits(c0)); accum = row max
    v = np.asarray(in0, np.float32)
    np_ = v.shape[0]
    vb = v.view(np.uint32).reshape(np_, -1)
    ib = np.asarray(in1, np.float32).view(np.uint32).reshape(np_, -1)
    c0f = np.float32(c0.flat[0] if isinstance(c0, np.ndarray) else c0)
    c1f = np.float32(c1.flat[0] if isinstance(c1, np.ndarray) else c1)
    mask = c0f.view(np.uint32)
    packed = (vb | (ib & mask)).view(np.float32)
    acc = np.maximum(packed.max(axis=-1, keepdims=True), c1f)
    return packed, acc


ARGMAX_PACK_ANT = _dve_ops.DveOp(
    "ARGMAX_PACK_ANT",
    _Spec(
        body=_Bin(_AluOp.BITWISE_OR, _Src0, _Bin(_AluOp.BITWISE_AND, _Src1, _C0)),
        accum=_maxx,
        accum_init=_C1,
        reference=_ref_argmax_pack,
    ),
    subdim=False,
    uops_sha={"v3": "1ec944e8e2fafb91", "v4": "a87bc82f01e7f970"},
)
if ARGMAX_PACK_ANT.name not in _dve_ops._SUB_OPCODE_FOR_NAME:
    _dve_ops.OPS.append(ARGMAX_PACK_ANT)
    _dve_ops.CUSTOM_DVE_SPECS[ARGMAX_PACK_ANT.name] = ARGMAX_PACK_ANT.spec
    _dve_ops._SUB_OPCODE_FOR_NAME[ARGMAX_PACK_ANT.name] = (
        max(_dve_ops._SUB_OPCODE_FOR_NAME.values()) + 1
    )

B = 8          # clouds
P = 2048       # points per cloud
DF = 16        # feature dim
NCH = P // 128   # 16 pred chunks of 128

f16 = mybir.dt.float16
f32 = mybir.dt.float32
u32 = mybir.dt.uint32

# ptabs column layout (x and pf duplicated so the A+B candidate
# recompute stages are single wide DVE ops against interleaved gall)
XC0 = 0                    # [128, 6*NCH] pred coords, duplicated [x|x]
PF0 = 6 * NCH              # [128, 2*DF*NCH] pred feats, duplicated [pf|pf]
NX0 = PF0 + 2 * DF * NCH   # [128, NCH] -|x|^2
PTW = NX0 + NCH            # total width (624)

_CACHED = {}


def _build_nc():
    nc = bacc.Bacc("TRN2", target_bir_lowering=False, debug=False, num_devices=B)

    xyaug = nc.dram_tensor("xyaug", [16, 2 * P], f16, kind="ExternalInput").ap()
    ptabs = nc.dram_tensor("ptabs", [128, PTW], f32, kind="ExternalInput").ap()
    ttab2 = nc.dram_tensor("ttab2", [P // 2, 40], f32, kind="ExternalInput").ap()
    iob = nc.dram_tensor("iob", [128, P // 2], f32, kind="ExternalInput").ap()
    res = nc.dram_tensor("res", [128, 4], f32, kind="ExternalOutput").ap()

    AL = mybir.AluOpType
    ACTF = mybir.ActivationFunctionType

    with tile.TileContext(nc) as tc:
        with (
            tc.tile_pool(name="const", bufs=1) as cpool,
            tc.tile_pool(name="d2", bufs=NCH) as d2pool,
            tc.tile_pool(name="tree", bufs=2) as tpool,
            tc.tile_pool(name="psmm", bufs=2, space="PSUM") as psmm,
        ):
            xyaug_s = cpool.tile([16, 2 * P], f16, tag="xyaug")
            ptabs_s = cpool.tile([128, PTW], f32, tag="ptabs")
            cm = cpool.tile([128, P], f16, tag="cm")
            m2 = cpool.tile([128, P], f16, tag="m2")
            rowA = cpool.tile([128, P], f16, tag="rowA")
            rowB = cpool.tile([128, P], f16, tag="rowB")
            rowC = cpool.tile([1, P], f16, tag="rowC")
            rjunk = cpool.tile([1, P], f16, tag="rjunk")
            iob_s = cpool.tile([128, P // 2], f32, tag="iob")
            packed_all = cpool.tile([128, NCH], f32, tag="packed")
            gall = cpool.tile([128, 40 * NCH], f32, tag="gall")
            cd = cpool.tile([128, 6 * NCH], f32, tag="cd")
            fd = cpool.tile([128, 2 * DF * NCH], f32, tag="fd")
            dmin2 = cpool.tile([128, 2 * NCH], f32, tag="dmin2")
            fsq2 = cpool.tile([128, 2 * NCH], f32, tag="fsq2")
            dmin = cpool.tile([128, NCH], f32, tag="dmin")
            fselA = cpool.tile([128, NCH], f32, tag="fselA")
            fmask = cpool.tile([128, NCH], f32, tag="fmask")
            idx2 = cpool.tile([128, NCH], u32, tag="idx2")
            dx = cpool.tile([128, 1], f32, tag="dx")
            df_ = cpool.tile([128, 1], f32, tag="df")
            fin = cpool.tile([128, 4], f32, tag="fin")
            junk_d = cpool.tile([128, 1], f32, tag="junk_d")
            junk_a = cpool.tile([128, 1], f32, tag="junk_a")
            idx0 = cpool.tile([128, 1], u32, tag="idx0")
            gjunk = cpool.tile([128, 40], f32, tag="gjunk")

            # --- input loads (single DMA each => single HW queue each) ---
            nc.sync.dma_start(xyaug_s[:, :], xyaug[:, :])
            nc.sync.dma_start(ptabs_s[:, :], ptabs[:, :])
            nc.sync.dma_start(iob_s[:, :], iob[:, :])
            nc.vector.memset(fin[:, :], 0.0)

            # sacrificial first indirect DMA: the first SWDGE descriptor
            # reads a stale offset on HW, so burn it on a dummy gather.
            nc.vector.memset(idx0[:, :], 0)
            nc.gpsimd.indirect_dma_start(
                out=gjunk[:, :],
                out_offset=None,
                in_=ttab2[:, :],
                in_offset=bass.IndirectOffsetOnAxis(ap=idx0[:, :], axis=0),
            )

            # observers: absorb input-DMA deps one engine at a time
            nc.vector.tensor_copy(out=junk_d[:, :], in_=ptabs_s[:, 0:1])
            nc.vector.tensor_copy(out=junk_d[:, :], in_=iob_s[:, 0:1])
            nc.scalar.activation(
                junk_a[:, :], ptabs_s[:, NX0 : NX0 + 1], ACTF.Copy, bias=0.0, scale=1.0
            )

            xc6 = ptabs_s[:, XC0 : XC0 + 6 * NCH].rearrange("p (c k) -> p c k", k=6)
            pf6 = ptabs_s[:, PF0 : PF0 + 2 * DF * NCH].rearrange(
                "p (c k) -> p c k", k=2 * DF
            )
            g3 = gall[:, :].rearrange("p (c k) -> p c k", k=40)
            cd6 = cd[:, :].rearrange("p (c k) -> p c k", k=6)
            cd4 = cd[:, :].rearrange("p (c n k) -> p c n k", n=2, k=3)
            dm3 = dmin2[:, :].rearrange("p (c n) -> p c n", n=2)
            fd6 = fd[:, :].rearrange("p (c k) -> p c k", k=2 * DF)
            fd4 = fd[:, :].rearrange("p (c n k) -> p c n k", n=2, k=DF)
            fq3 = fsq2[:, :].rearrange("p (c n) -> p c n", n=2)

            def recompute(lo, hi):
                sl = slice(lo, hi)
                nc.vector.tensor_tensor(
                    out=cd6[:, sl], in0=xc6[:, sl], in1=g3[:, sl, 0:6],
                    op=AL.subtract,
                )
                nc.vector.tensor_tensor(
                    out=cd[:, 6 * lo : 6 * hi], in0=cd[:, 6 * lo : 6 * hi],
                    in1=cd[:, 6 * lo : 6 * hi], op=AL.mult,
                )
                nc.vector.tensor_reduce(
                    out=dm3[:, sl], in_=cd4[:, sl],
                    axis=mybir.AxisListType.X, op=AL.add,
                )
                nc.vector.tensor_tensor(
                    out=fd6[:, sl], in0=pf6[:, sl], in1=g3[:, sl, 6 : 6 + 2 * DF],
                    op=AL.subtract,
                )
                nc.vector.tensor_tensor(
                    out=fd[:, 2 * DF * lo : 2 * DF * hi],
                    in0=fd[:, 2 * DF * lo : 2 * DF * hi],
                    in1=fd[:, 2 * DF * lo : 2 * DF * hi], op=AL.mult,
                )
                nc.vector.tensor_reduce(
                    out=fq3[:, sl], in_=fd4[:, sl],
                    axis=mybir.AxisListType.X, op=AL.add,
                )


            # --- main loop over pred chunks ---
            for c in range(NCH):
                ps = psmm.tile([128, P], f32, tag="mm")
                for t in range(4):
                    nc.tensor.matmul(
                        ps[:, 512 * t : 512 * (t + 1)],
                        lhsT=xyaug_s[:, bass.ts(c, 128)],
                        rhs=xyaug_s[:, P + 512 * t : P + 512 * (t + 1)],
                        start=True,
                        stop=True,
                    )
                d2c = d2pool.tile([128, P], f16, tag="d2")
                # neg_d2 = (2xy - |y|^2) - |x|^2, one wide cast to fp16
                nc.scalar.activation(
                    d2c[:, :],
                    ps[:, :],
                    ACTF.Identity,
                    bias=ptabs_s[:, NX0 + c : NX0 + c + 1],
                    scale=1.0,
                )
                if c == 15:
                    # m2 halves RIGHT AFTER cast15 (ACT runs far ahead of
                    # DVE) so the AR#2 halves clear Pool before the last
                    # gather needs it
                    d2c14 = d2c14_ref[0]
                    nc.vector.tensor_tensor(
                        out=m2[:, 0:1024], in0=d2c14[:, 0:1024],
                        in1=d2c[:, 0:1024], op=AL.max,
                    )
                    nc.gpsimd.partition_all_reduce(
                        rowB[:, 0:1024], m2[:, 0:1024], 128, _ReduceOp.max
                    )
                    nc.vector.tensor_tensor(
                        out=m2[:, 1024:2048], in0=d2c14[:, 1024:2048],
                        in1=d2c[:, 1024:2048], op=AL.max,
                    )
                    nc.gpsimd.partition_all_reduce(
                        rowB[:, 1024:2048], m2[:, 1024:2048], 128, _ReduceOp.max
                    )
                # pre-reduce halves at 2x, then packed argmax over 1024
                # cols; the j / j+1024 ambiguity is resolved by gathering
                # BOTH candidates (interleaved in one ttab2 row) and
                # letting the exact fp32 recompute pick.
                t1 = tpool.tile([128, 1024], f16, tag="t1")
                nc.vector.tensor_tensor(
                    out=t1[:, :], in0=d2c[:, 0:1024], in1=d2c[:, 1024:2048], op=AL.max
                )
                pk = tpool.tile([128, 1024], f32, tag="pk")
                nc.vector._custom_dve(
                    ARGMAX_PACK_ANT,
                    out=pk[:, :],
                    in0=t1[:, :],
                    in1=iob_s[:, :],
                    s0=IDX_MASK_F,
                    s1=NEG_HUGE,
                    accum_out=packed_all[:, c : c + 1],
                )
                # column-direction running elementwise max: chunks 0..13
                # accumulate into cm (partition-reduced early, while the
                # loop tail runs); 14/15 into m2 (reduced separately).
                if c == 0:
                    nc.vector.tensor_copy(out=cm[:, :], in_=d2c[:, :])
                elif c <= 13:
                    nc.vector.tensor_tensor(
                        out=cm[:, :], in0=cm[:, :], in1=d2c[:, :], op=AL.max
                    )
                elif c == 14:
                    d2c14_ref = [d2c]

                if c == 13:
                    nc.gpsimd.partition_all_reduce(
                        rowA[:, :], cm[:, :], 128, _ReduceOp.max
                    )
                # per-chunk index extraction + single gather (ttab2 rows
                # carry BOTH candidates; HW SWDGE honors one index per
                # partition per gather, so batching across chunks is out)
                nc.vector.tensor_scalar(
                    out=idx2[:, c : c + 1],
                    in0=packed_all[:, c : c + 1].bitcast(u32),
                    scalar1=IDX_MASK_BITS,
                    scalar2=None,
                    op0=AL.bitwise_and,
                )
                nc.gpsimd.indirect_dma_start(
                    out=gall[:, 40 * c : 40 * (c + 1)],
                    out_offset=None,
                    in_=ttab2[:, :],
                    in_offset=bass.IndirectOffsetOnAxis(ap=idx2[:, c : c + 1], axis=0),
                )

            # --- x-direction: recompute BOTH candidates in fp32, pick min.
            # ttab2 interleaves candidate columns (yA|yB|tfA|tfB) and ptabs
            # duplicates x/pf, so each A+B stage is ONE wide DVE op. Split
            # into chunks 0..13 / 14..15 so the big half doesn't wait for
            # the last gather, and interleave the rowC/ysum halves (each
            # waits on its AR#2 half) between recompute stages.
            recompute(0, 14)
            # rowC halves at 2x; free-axis sums on the idle ACT engine, the
            # two half-sums land on different fin partitions so the final
            # partition-reducing matmul adds them for free.
            nc.vector.tensor_tensor(
                out=rowC[:, 0:1024], in0=rowA[0:1, 0:1024],
                in1=rowB[0:1, 0:1024], op=AL.max,
            )
            nc.scalar.activation(
                rjunk[:, 0:1024], rowC[:, 0:1024], ACTF.Identity,
                bias=0.0, scale=1.0, accum_out=fin[0:1, 1:2],
            )
            nc.vector.tensor_tensor(
                out=rowC[:, 1024:2048], in0=rowA[0:1, 1024:2048],
                in1=rowB[0:1, 1024:2048], op=AL.max,
            )
            nc.scalar.activation(
                rjunk[:, 1024:2048], rowC[:, 1024:2048], ACTF.Identity,
                bias=0.0, scale=1.0, accum_out=fin[0:1, 3:4],
            )
            recompute(14, 16)
            distA, distB = dm3[:, :, 0:1], dm3[:, :, 1:2]
            fqA, fqB = fq3[:, :, 0:1], fq3[:, :, 1:2]
            nc.vector.tensor_reduce(
                out=dmin[:, :], in_=dm3, axis=mybir.AxisListType.X, op=AL.min
            )
            nc.vector.tensor_reduce(
                out=dx[:, :], in_=dmin[:, :], axis=mybir.AxisListType.X, op=AL.add
            )
            # fsel = fsqB + (fsqA - fsqB) * (distA <= distB)
            fmask3 = fmask[:, :].rearrange("p (c n) -> p c n", n=1)
            fselA3 = fselA[:, :].rearrange("p (c n) -> p c n", n=1)
            nc.vector.tensor_tensor(out=fmask3, in0=distA, in1=distB, op=AL.is_le)
            nc.vector.tensor_tensor(out=fselA3, in0=fqA, in1=fqB, op=AL.subtract)
            nc.vector.tensor_tensor(
                out=fselA[:, :], in0=fselA[:, :], in1=fmask[:, :], op=AL.mult
            )
            nc.vector.tensor_tensor(out=fselA3, in0=fselA3, in1=fqB, op=AL.add)
            nc.vector.tensor_reduce(
                out=df_[:, :], in_=fselA[:, :], axis=mybir.AxisListType.X, op=AL.add
            )

            # --- stack per-partition partials; host does the final sums ---
            nc.vector.tensor_copy(out=fin[:, 0:1], in_=dx[:, :])
            nc.vector.tensor_copy(out=fin[:, 2:3], in_=df_[:, :])
            nc.sync.dma_start(res[:, :], fin[:, :])

    nc.compile()
    return nc


def _prep_core(x, y, pf, tf):
    """Host-side layout prep for one cloud (dtype splits / transposes)."""
    x = np.ascontiguousarray(x, np.float32)
    y = np.ascontiguousarray(y, np.float32)
    xh = x.astype(np.float16)
    xl = (x - xh.astype(np.float32)).astype(np.float16)
    yh = y.astype(np.float16)
    yl = (y - yh.astype(np.float32)).astype(np.float16)

    ny2 = (y.astype(np.float64) ** 2).sum(1)
    a0 = (-ny2).astype(np.float16)
    r = -ny2 - a0.astype(np.float64)
    a1 = r.astype(np.float16)
    a2 = (r - a1.astype(np.float64)).astype(np.float16)

    xyaug = np.zeros((16, 2 * P), np.float16)
    for k in range(3):
        txh = (xh[:, k].astype(np.float32) * 2).astype(np.float16)
        txl = (xl[:, k].astype(np.float32) * 2).astype(np.float16)
        xyaug[4 * k + 0, :P] = txh
        xyaug[4 * k + 1, :P] = txh
        xyaug[4 * k + 2, :P] = txl
        xyaug[4 * k + 3, :P] = txl
        xyaug[4 * k + 0, P:] = yh[:, k]
        xyaug[4 * k + 1, P:] = yl[:, k]
        xyaug[4 * k + 2, P:] = yh[:, k]
        xyaug[4 * k + 3, P:] = yl[:, k]
    xyaug[12:15, :P] = np.float16(1.0)
    xyaug[12, P:] = a0
    xyaug[13, P:] = a1
    xyaug[14, P:] = a2

    nx2 = (x.astype(np.float64) ** 2).sum(1).astype(np.float32)

    ptabs = np.zeros((128, PTW), np.float32)
    xcd = x.reshape(NCH, 128, 3)
    ptabs[:, XC0 : XC0 + 6 * NCH] = (
        np.concatenate([xcd, xcd], axis=2).transpose(1, 0, 2).reshape(128, 6 * NCH)
    )
    pfc = np.asarray(pf, np.float32).reshape(NCH, 128, DF)
    ptabs[:, PF0 : PF0 + 2 * DF * NCH] = (
        np.concatenate([pfc, pfc], axis=2)
        .transpose(1, 0, 2).reshape(128, 2 * DF * NCH)
    )
    ptabs[:, NX0 : NX0 + NCH] = (-nx2).reshape(NCH, 128).T

    # interleaved candidate table: row j = [y_j | y_{j+1024} | tf_j |
    # tf_{j+1024} | pad] so one gather returns both argmax candidates and
    # the A+B recompute stages are single wide ops
    ttab2 = np.zeros((P // 2, 40), np.float32)
    ttab2[:, 0:3] = y[: P // 2]
    ttab2[:, 3:6] = y[P // 2 :]
    ttab2[:, 6 : 6 + DF] = tf[: P // 2]
    ttab2[:, 6 + DF : 6 + 2 * DF] = tf[P // 2 :]

    iob = (np.uint32(0x3F800000) | np.arange(P // 2, dtype=np.uint32)).view(
        np.float32
    )
    iob = np.broadcast_to(iob, (128, P // 2)).copy()
    return {"xyaug": xyaug, "ptabs": ptabs, "ttab2": ttab2, "iob": iob}


def kernel(pred_coord, target_coord, pred_feat, target_feat,
           pred_offset, target_offset):
    pred_offset = np.asarray(pred_offset)
    target_offset = np.asarray(target_offset)
    starts_p = np.concatenate([[0], pred_offset[:-1]])
    starts_t = np.concatenate([[0], target_offset[:-1]])
    assert np.all(pred_offset - starts_p == P), "kernel hardcodes equal segments"
    assert np.all(target_offset - starts_t == P), "kernel hardcodes equal segments"

    if "nc" not in _CACHED:
        _CACHED["nc"] = _build_nc()
    nc = _CACHED["nc"]

    in_maps = []
    for b in range(B):
        sp, st = int(starts_p[b]), int(starts_t[b])
        in_maps.append(
            _prep_core(
                np.asarray(pred_coord)[sp : sp + P],
                np.asarray(target_coord)[st : st + P],
                np.asarray(pred_feat)[sp : sp + P],
                np.asarray(target_feat)[st : st + P],
            )
        )

    out = run_bass_kernel_spmd(nc, in_maps, core_ids=list(range(B)))
    rs = np.stack([out.results[b]["res"] for b in range(B)])  # [B, 128, 4]

    sum_x = rs[:, :, 0].sum(1)  # per-cloud sum of recomputed nearest dists
    sum_y = -(rs[:, 0, 1] + rs[:, 0, 3])  # min-dist sum (tgt->pred), negated halves
    sum_f = rs[:, :, 2].sum(1)  # per-cloud sum of squared feature diffs

    cham_x = sum_x / np.float32(P)
    cham_y = sum_y / np.float32(P)
    coord_loss = np.float32((cham_x + cham_y).sum() / B)
    feat_loss = np.float32(sum_f.sum() / (B * P * DF))
    loss = np.float32(1.0) * (np.float32(1.0) * coord_loss + np.float32(0.1) * feat_loss)
    return (np.float32(loss), np.float32(coord_loss), np.float32(feat_loss))
